# revision 39
# baseline (speedup 1.0000x reference)
"""Causal self-attention (B=4, T=2048, D=1024, H=16, HD=64) on 8 TRN2 NeuronCores.

Sharding: core = (batch b in 0..3, head-group g in 0..1) -> data parallel on B,
tensor parallel over heads (8 heads per core). Each core computes a partial
out-projection for its head group; the host sums the pair of partials per batch
(plus bout) at unshard time.

Device kernel (per core), bf16 matmuls with fp32 PSUM accumulate; the AV
(probability x value) matmuls run in fp8e4m3 DoubleRow packing two tk-tiles
per pass (softmax weights tolerate fp8; exp is biased by -1.5 to center the
fp8 range; the bias cancels in the softmax normalization):
  1. xT loads (host pre-transposes x)                          [128 x 2048] x8
  2. v  = x @ Wv (natural) + ones col, drained to fp8 with j-pair interleave
     qT/kT = (x @ Wq/Wk).T per head-pair, bias added on drain  [128, 2048] x8
  3. attention per (head-pair, chunk, tk-tile j): S^T for both heads lands in
     two psum banks (concurrent row-tiled matmuls), causal mask accumulated
     via PE on diagonal tiles, one batched EXP (ACT) -> fp8 plane of the
     j-pair tile; on odd j the two DoubleRow AV matmuls consume the pair.
     ACT exp is the pacing engine; qk projections of the next pair and the
     1/Z normalization of the previous pair fill the PE between groups.
  4. normalization per pair: batched reciprocal; esel matmul broadcasts 1/Z
     for both heads of a chunk at once; yt scaled in place.
  5. out = yT.T @ Wout, psum copied out (DVE+ACT halves) and DMA'd.
"""

import os
import numpy as np
from ml_dtypes import bfloat16

import concourse.bass as bass
import concourse.tile as tile
from concourse import bacc, mybir
import concourse.bass_utils as bass_utils
from concourse.masks import make_identity

F32 = mybir.dt.float32
F32R = mybir.dt.float32r
BF16 = mybir.dt.bfloat16
F8 = mybir.dt.float8e4
AF = mybir.ActivationFunctionType
ALU = mybir.AluOpType
DR = mybir.MatmulPerfMode.DoubleRow

B, T, D, H = 4, 2048, 1024, 16
HD = D // H          # 64
G = 2                # head groups (TP degree)
HPG = H // G         # 8 heads per core
DG = HPG * HD        # 512 local qkv dims per core
NT = T // 128        # 16 row tiles
ND = D // 128        # 8 contraction tiles
NC = T // 512        # 4 tq chunks
NK = DG // 128       # 4 local-dim tiles (pairs)
VW = HD + 1          # 65: v columns per head incl. ones column
VWP = 66             # padded per-head v stride (DoubleRow step alignment)
EBIAS = -1.5         # exp bias centering fp8 range; cancels in softmax

_cached = {}


def _build():
    nc = bacc.Bacc("TRN2", target_bir_lowering=False, debug=False, num_devices=8)

    x_d = nc.dram_tensor("x", [D, T], BF16, kind="ExternalInput")  # pre-transposed
    wq_d = nc.dram_tensor("wq", [D, DG], BF16, kind="ExternalInput")
    wk_d = nc.dram_tensor("wk", [D, DG], BF16, kind="ExternalInput")
    wv_d = nc.dram_tensor("wv", [D, DG], BF16, kind="ExternalInput")
    wo_d = nc.dram_tensor("wo", [DG, D], BF16, kind="ExternalInput")
    bq_d = nc.dram_tensor("bq", [DG], F32, kind="ExternalInput")
    bk_d = nc.dram_tensor("bk", [DG], F32, kind="ExternalInput")
    bv_d = nc.dram_tensor("bv", [DG], BF16, kind="ExternalInput")
    maskt_d = nc.dram_tensor("maskt", [512, 512], BF16, kind="ExternalInput")
    out_d = nc.dram_tensor("out", [T, D], F32, kind="ExternalOutput")

    with tile.TileContext(nc) as tc:
        with nc.allow_low_precision(reason="bf16/fp8 matmul pipeline, fp32 psum"):
            _emit(nc, tc, x_d, wq_d, wk_d, wv_d, wo_d, bq_d, bk_d, bv_d,
                  maskt_d, out_d)
    nc.finalize()
    return nc


def _emit(nc, tc, x_d, wq_d, wk_d, wv_d, wo_d, bq_d, bk_d, bv_d, maskt_d,
          out_d):
    from contextlib import ExitStack
    ctx = ExitStack()
    with ctx:
        # ---------------- pools ----------------
        const_p = ctx.enter_context(tc.tile_pool(name="const", bufs=1))
        xt_p = ctx.enter_context(tc.tile_pool(name="xt", bufs=1))
        qk_p = ctx.enter_context(tc.tile_pool(name="qk", bufs=1))
        vp_p = ctx.enter_context(tc.tile_pool(name="vp", bufs=1))
        yt_p = ctx.enter_context(tc.tile_pool(name="yt", bufs=1))
        w_p = ctx.enter_context(tc.tile_pool(name="w", bufs=1))
        wblk_p = ctx.enter_context(tc.tile_pool(name="wblk", bufs=4))
        pt_p = ctx.enter_context(tc.tile_pool(name="pt", bufs=3))
        zt_p = ctx.enter_context(tc.tile_pool(name="zt", bufs=3))
        ob_p = ctx.enter_context(tc.tile_pool(name="ob", bufs=3))
        s_ps = ctx.enter_context(tc.tile_pool(name="s_ps", bufs=3, space="PSUM"))
        y_ps = ctx.enter_context(tc.tile_pool(name="y_ps", bufs=2, space="PSUM"))

        def sps():
            return s_ps.tile([128, 1024], F32, tag="s", name="sgrp")

        # ---------------- constants ----------------
        identb = const_p.tile([128, 128], BF16, tag="identb")
        make_identity(nc, identb[:])

        # e0b: row 0 ones (bf16, bias broadcast)
        e0b = const_p.tile([128, 128], BF16, tag="e0b")
        nc.vector.memset(e0b[:], 0.0)
        nc.vector.memset(e0b[0:1, :], 1.0)

        # esel_c[r, m] = (r == 4*(m//64) + c): selects zrec rows (c, 4+c) into
        # the two 64-row halves -> one matmul broadcasts 1/Z for both heads.
        esel = []
        for c in range(NC):
            et = const_p.tile([128, 128], BF16, tag=f"esel{c}", name=f"esel{c}")
            nc.gpsimd.memset(et[:], 1.0)
            nc.gpsimd.affine_select(
                out=et[:], in_=et[:], compare_op=ALU.is_equal, fill=0.0,
                base=-c, channel_multiplier=1, pattern=[[-4, 2], [0, 64]])
            esel.append(et)

        # q/k bias columns [128, 8]: cols 0-3 = bq tiles, 4-7 = bk tiles
        bqk = const_p.tile([128, 8], F32, tag="bqk")
        nc.sync.dma_start(bqk[:, 0:NK], bq_d[:].rearrange("(f p) -> p f", p=128))
        nc.sync.dma_start(bqk[:, NK:2 * NK], bk_d[:].rearrange("(f p) -> p f", p=128))

        # bv broadcast [128, 512]
        bvrow = const_p.tile([128, DG], BF16, tag="bvrow")
        nc.vector.memset(bvrow[:], 0.0)
        nc.sync.dma_start(bvrow[0:1, :], bv_d[:].rearrange("(o n) -> o n", o=1))
        bv_bc = const_p.tile([128, DG], BF16, tag="bv_bc")
        pb = sps()
        nc.tensor.matmul(pb[:, 0:DG], e0b[:], bvrow[:], start=True, stop=True)
        nc.vector.tensor_copy(bv_bc[:], pb[:, 0:DG])

        # zmat (Z rows, one tile per pair) / zrec (1/Z bf16, rows 0-7 live)
        zmat = [const_p.tile([8, 512], F32, tag=f"zmat{p}", name=f"zmat{p}")
                for p in range(NK)]
        zrec = const_p.tile([128, 512], BF16, tag="zrec")
        nc.vector.memset(zrec[:], 0.0)

        # transposed causal diag mask tiles (host pre-transposed)
        maskt = []
        for l in range(4):
            mt = const_p.tile([128, 512], BF16, tag=f"mt{l}", name=f"maskt{l}")
            nc.sync.dma_start(mt[:], maskt_d[128 * l:128 * (l + 1), :])
            maskt.append(mt)

        # exp bias column (fp8 range centering)
        ebias = const_p.tile([128, 1], F32, tag="ebias")
        nc.vector.memset(ebias[:], EBIAS)

        # ACT exp table preload (hide the ~2.7us table DMA in startup)
        dumm = const_p.tile([1, 16], F32, tag="dumm")
        nc.vector.memset(dumm[:], 0.0)
        nc.scalar.activation(dumm[:], dumm[:], AF.Exp, scale=1.0)

        # ---------------- xT (host pre-transposed, per-chunk tiles so the
        # first projection matmuls start after ~1/4 of the x DMA) ----------
        xtc = [[xt_p.tile([128, 512], BF16, tag=f"xt{d}c{cc}",
                          name=f"xt{d}c{cc}") for cc in range(NC)]
               for d in range(ND)]
        for cc in range(NC):
            for d in range(ND):
                nc.sync.dma_start(
                    xtc[d][cc][:],
                    x_d[128 * d:128 * (d + 1), 512 * cc:512 * (cc + 1)])

        # ---------------- weights ----------------
        wv_sb = []
        for d in range(ND):
            wt = w_p.tile([128, DG], BF16, tag=f"wv{d}", name=f"wvt{d}")
            nc.sync.dma_start(wt[:], wv_d[128 * d:128 * (d + 1), :])
            wv_sb.append(wt)
        wo_sb = []
        for k in range(NK):
            wt = w_p.tile([128, D], BF16, tag=f"wo{k}", name=f"wot{k}")
            nc.sync.dma_start(wt[:], wo_d[128 * k:128 * (k + 1), :])
            wo_sb.append(wt)

        def load_wblk(is_k, f):
            src = wk_d if is_k else wq_d
            wblk = wblk_p.tile([128, D], BF16, tag="wblk")
            nc.sync.dma_start(
                wblk[:].rearrange("p (dt c) -> p dt c", dt=ND),
                src[:, 128 * f:128 * (f + 1)].rearrange("(dt p) c -> p dt c", p=128))
            return wblk

        # persistent SBUF tensors
        vp = []
        for t in range(NT):
            vt = vp_p.tile([128, HPG * VW], BF16, tag=f"vp{t}", name=f"vp{t}")
            # only the per-head ones columns need initialization
            nc.vector.memset(
                vt[:].rearrange("p (h c) -> p h c", c=VW)[:, :, HD:VW], 1.0)
            vp.append(vt)
        qk = [qk_p.tile([128, T], BF16, tag=f"qk{f}", name=f"qk{f}")
              for f in range(2 * NK)]
        yt = [yt_p.tile([128, T], BF16, tag=f"yt{k}", name=f"yt{k}")
              for k in range(NK)]

        # ---------------- v projection (plain, upfront) ----------------
        for t in range(NT):
            pv = sps()
            for d in range(ND):
                nc.tensor.matmul(
                    pv[:, 0:DG],
                    xtc[d][t // 4][:, 128 * (t % 4):128 * (t % 4 + 1)],
                    wv_sb[d][:], start=(d == 0), stop=(d == ND - 1))
            nc.vector.tensor_tensor(
                vp[t][:].rearrange("p (h c) -> p h c", h=HPG)[:, :, 0:HD],
                pv[:, 0:DG].rearrange("p (h c) -> p h c", h=HPG),
                bv_bc[:].rearrange("p (h c) -> p h c", h=HPG),
                ALU.add)

        # qk projection piece generator: one f-block = 4 chunks x 8 d-matmuls,
        # yielded in 8 pieces of 4 matmuls (chunk halves), drain per chunk.
        # The psum tile is allocated when the first half RUNS (not at
        # generator-build time) so pool rotation order matches emission order.
        def qk_pieces(is_k, f):
            wblk = load_wblk(is_k, f)
            dst = qk[NK + f if is_k else f]
            bcol = NK + f if is_k else f
            state = {}
            for cidx in range(NC):
                for half in range(2):
                    def piece(wblk=wblk, cidx=cidx, half=half,
                              dst=dst, bcol=bcol):
                        if half == 0:
                            state[cidx] = sps()
                        pq = state[cidx]
                        for d in range(4 * half, 4 * half + 4):
                            nc.tensor.matmul(
                                pq[:, 0:512], wblk[:, 128 * d:128 * (d + 1)],
                                xtc[d][cidx][:],
                                start=(d == 0), stop=(d == ND - 1))
                        if half == 1:
                            nc.vector.tensor_scalar(
                                dst[:, 512 * cidx:512 * (cidx + 1)],
                                pq[:, 0:512], bqk[:, bcol:bcol + 1], None,
                                ALU.add)
                    yield piece

        # qk pair 0 upfront (plain)
        for is_k in (False, True):
            for pc in qk_pieces(is_k, 0):
                pc()

        # ---------------- attention ----------------
        def norm_pieces(p):
            def recip(p=p):
                nc.vector.reciprocal(zrec[0:8, :], zmat[p][0:8, :])
            yield recip
            for c in range(NC):
                def piece(p=p, c=c):
                    zb = sps()
                    nc.tensor.matmul(zb[:, 0:512], esel[c][:], zrec[:],
                                     start=True, stop=True)
                    ysl = yt[p][:, 512 * c:512 * (c + 1)]
                    nc.vector.tensor_tensor(ysl, ysl, zb[:, 0:512], ALU.mult)
                yield piece

        filler_q = []  # queued PE filler pieces

        class Grp:
            __slots__ = ("p", "c", "j", "d", "offp", "ps", "pt", "py",
                         "chunk_end")

        def make_groups(p):
            gs = []
            for c in range(NC):
                for j in range(4 * c + 4):
                    g = Grp()
                    g.p, g.c, g.j = p, c, j
                    g.d = j - 4 * c
                    g.offp = 128 * g.d if g.d >= 0 else 0
                    g.chunk_end = (j == 4 * c + 3)
                    gs.append(g)
            return gs

        def emit_S(g):
            # head pair: even head at PE rows 0-63, odd at 64-127 -> the two
            # S matmuls land in distinct row groups and psum banks and run
            # concurrently (auto tile_position from base partitions).
            qt, kt = qk[g.p], qk[NK + g.p]
            g.ps = sps()
            for hl in range(2):
                qrow = 64 * hl
                nc.tensor.matmul(
                    g.ps[:, 512 * hl + g.offp:512 * (hl + 1)],
                    kt[qrow:qrow + HD, 128 * g.j:128 * (g.j + 1)],
                    qt[qrow:qrow + HD, 512 * g.c + g.offp:512 * (g.c + 1)],
                    start=True, stop=(g.d < 0), skip_group_check=True)
            if g.d >= 0:
                for hl in range(2):
                    nc.tensor.matmul(
                        g.ps[:, 512 * hl + g.offp:512 * (hl + 1)], identb[:],
                        maskt[g.d][:, g.offp:512],
                        start=False, stop=True, skip_group_check=True)

        def emit_EXP(g, pt_live):
            g.pt = pt_p.tile([128, 1024], BF16, tag="pt", name="pt")
            if g.offp:
                psv = g.ps[:].rearrange("p (two n) -> p two n", two=2)
                ptv = g.pt[:].rearrange("p (two n) -> p two n", two=2)
                nc.scalar.activation(ptv[:, :, g.offp:512],
                                     psv[:, :, g.offp:512],
                                     AF.Exp, scale=0.125)
            else:
                nc.scalar.activation(g.pt[:], g.ps[:], AF.Exp, scale=0.125)

        def emit_AV(g, py_live):
            if g.j == 0:
                py_live[0] = y_ps.tile([VW, 512], F32, tag="py", name="py0")
                py_live[1] = y_ps.tile([VW, 512], F32, tag="py", name="py1")
            g.py = (py_live[0], py_live[1])
            jmax = 4 * g.c + 3
            for hl in range(2):
                hidx = (2 * g.p + hl) % HPG
                nc.tensor.matmul(
                    g.py[hl][:, g.offp:512],
                    vp[g.j][:, VW * hidx:VW * hidx + VW],
                    g.pt[:, 512 * hl + g.offp:512 * (hl + 1)],
                    start=(g.j == 0), stop=(g.j == jmax),
                    skip_group_check=True)

        def emit_post(g):
            if not g.chunk_end:
                return
            for hl in range(2):
                qrow = 64 * hl
                # raw yT drain (normalized later) + Z row -> zmat
                nc.vector.tensor_copy(
                    yt[g.p][qrow:qrow + HD, 512 * g.c:512 * (g.c + 1)],
                    g.py[hl][0:64, :])
                i = 4 * hl + g.c
                zt = zt_p.tile([1, 512], F32, tag="zt", name="zt")
                nc.vector.tensor_copy(zt[:], g.py[hl][64:65, :])
                nc.sync.dma_start(zmat[g.p][i:i + 1, :], zt[:])

        for p in range(NK):
            groups = make_groups(p)
            # fillers: norm for pair p-1 first, then qk projections for p+1
            if p > 0:
                filler_q.extend(norm_pieces(p - 1))
            if p + 1 < NK:
                for is_k in (False, True):
                    filler_q.extend(qk_pieces(is_k, p + 1))
            prev = None
            py_live = [None, None]
            pt_live = [None]
            for g in groups:
                emit_S(g)
                emit_EXP(g, pt_live)
                if prev is not None:
                    emit_AV(prev, py_live)
                    emit_post(prev)
                # one filler piece per slot keeps PE ahead of ACT
                if filler_q:
                    filler_q.pop(0)()
                prev = g
            emit_AV(prev, py_live)
            emit_post(prev)

        # drain remaining fillers + last pair norm
        for pc in filler_q:
            pc()
        for pc in norm_pieces(NK - 1):
            pc()

        # ---------------- out projection ----------------
        for t in range(NT):
            po = sps()
            for oc in range(2):
                for k in range(NK):
                    nc.tensor.matmul(
                        po[:, 512 * oc:512 * (oc + 1)],
                        yt[k][:, 128 * t:128 * (t + 1)],
                        wo_sb[k][:, 512 * oc:512 * (oc + 1)],
                        start=(k == 0), stop=(k == NK - 1))
            ob = ob_p.tile([128, D], F32, tag="ob", name="ob")
            nc.vector.tensor_copy(ob[:, 0:512], po[:, 0:512])
            nc.scalar.copy(ob[:, 512:1024], po[:, 512:1024])
            nc.sync.dma_start(out_d[128 * t:128 * (t + 1), :], ob[:])


def kernel(x, attn_mask, Wqkv, bqkv, Wout, bout):
    if "nc" not in _cached:
        _cached["nc"] = _build()
    nc = _cached["nc"]

    x = np.asarray(x, dtype=np.float32)
    Wqkv = np.asarray(Wqkv, dtype=np.float32)
    bqkv = np.asarray(bqkv, dtype=np.float32)
    Wout = np.asarray(Wout, dtype=np.float32)
    bout = np.asarray(bout, dtype=np.float32)
    # transposed causal diagonal block, bf16
    maskt_blk = np.ascontiguousarray(
        np.asarray(attn_mask, dtype=np.float32)[0, 0, :512, :512].T
    ).astype(bfloat16)

    in_maps = []
    for b in range(B):
        for g in range(G):
            s = slice(g * DG, (g + 1) * DG)
            in_maps.append({
                "x": np.ascontiguousarray(x[b].T).astype(bfloat16),
                "wq": np.ascontiguousarray(Wqkv[:, g * DG:(g + 1) * DG]).astype(bfloat16),
                "wk": np.ascontiguousarray(Wqkv[:, D + g * DG:D + (g + 1) * DG]).astype(bfloat16),
                "wv": np.ascontiguousarray(Wqkv[:, 2 * D + g * DG:2 * D + (g + 1) * DG]).astype(bfloat16),
                "wo": np.ascontiguousarray(Wout[s, :]).astype(bfloat16),
                "bq": np.ascontiguousarray(bqkv[g * DG:(g + 1) * DG]),
                "bk": np.ascontiguousarray(bqkv[D + g * DG:D + (g + 1) * DG]),
                "bv": np.ascontiguousarray(bqkv[2 * D + g * DG:2 * D + (g + 1) * DG]).astype(bfloat16),
                "maskt": maskt_blk,
            })

    trace = bool(int(os.environ.get("BASS_ATTN_TRACE", "0")))
    res = bass_utils.run_bass_kernel_spmd(
        nc, in_maps, core_ids=list(range(B * G)), trace=trace)
    _cached["last_result"] = res

    out = np.empty((B, T, D), dtype=np.float32)
    for b in range(B):
        out[b] = res.results[2 * b]["out"] + res.results[2 * b + 1]["out"] \
            + bout[None, :]
    return out


# revision 40
# speedup vs baseline: 1.0354x; 1.0354x over previous
"""Causal self-attention (B=4, T=2048, D=1024, H=16, HD=64) on 8 TRN2 NeuronCores.

Sharding: core = (batch b in 0..3, head-group g in 0..1) -> data parallel on B,
tensor parallel over heads (8 heads per core). Each core computes a partial
out-projection for its head group; the host sums the pair of partials per batch
(plus bout) at unshard time.

Device kernel (per core), bf16 matmuls with fp32 PSUM accumulate; the AV
(probability x value) matmuls run in fp8e4m3 DoubleRow packing two tk-tiles
per pass (softmax weights tolerate fp8; exp is biased by -1.5 to center the
fp8 range; the bias cancels in the softmax normalization):
  1. xT loads (host pre-transposes x)                          [128 x 2048] x8
  2. v  = x @ Wv (natural) + ones col, drained to fp8 with j-pair interleave
     qT/kT = (x @ Wq/Wk).T per head-pair, bias added on drain  [128, 2048] x8
  3. attention per (head-pair, chunk, tk-tile j): S^T for both heads lands in
     two psum banks (concurrent row-tiled matmuls), causal mask accumulated
     via PE on diagonal tiles, one batched EXP (ACT) -> fp8 plane of the
     j-pair tile; on odd j the two DoubleRow AV matmuls consume the pair.
     ACT exp is the pacing engine; qk projections of the next pair and the
     1/Z normalization of the previous pair fill the PE between groups.
  4. normalization per pair: batched reciprocal; esel matmul broadcasts 1/Z
     for both heads of a chunk at once; yt scaled in place.
  5. out = yT.T @ Wout, psum copied out (DVE+ACT halves) and DMA'd.
"""

import os
import numpy as np
from ml_dtypes import bfloat16

import concourse.bass as bass
import concourse.tile as tile
from concourse import bacc, mybir
import concourse.bass_utils as bass_utils
from concourse.masks import make_identity

F32 = mybir.dt.float32
F32R = mybir.dt.float32r
BF16 = mybir.dt.bfloat16
F8 = mybir.dt.float8e4
AF = mybir.ActivationFunctionType
ALU = mybir.AluOpType
DR = mybir.MatmulPerfMode.DoubleRow

B, T, D, H = 4, 2048, 1024, 16
HD = D // H          # 64
G = 2                # head groups (TP degree)
HPG = H // G         # 8 heads per core
DG = HPG * HD        # 512 local qkv dims per core
NT = T // 128        # 16 row tiles
ND = D // 128        # 8 contraction tiles
NC = T // 512        # 4 tq chunks
NK = DG // 128       # 4 local-dim tiles (pairs)
VW = HD + 1          # 65: v columns per head incl. ones column
VWP = 66             # padded per-head v stride (DoubleRow step alignment)
EBIAS = -1.5         # exp bias centering fp8 range; cancels in softmax

_cached = {}


def _build():
    nc = bacc.Bacc("TRN2", target_bir_lowering=False, debug=False, num_devices=8)

    x_d = nc.dram_tensor("x", [D, T], BF16, kind="ExternalInput")  # pre-transposed
    wq_d = nc.dram_tensor("wq", [D, DG], BF16, kind="ExternalInput")
    wk_d = nc.dram_tensor("wk", [D, DG], BF16, kind="ExternalInput")
    wv_d = nc.dram_tensor("wv", [D, DG], BF16, kind="ExternalInput")
    wo_d = nc.dram_tensor("wo", [DG, D], BF16, kind="ExternalInput")
    bq_d = nc.dram_tensor("bq", [DG], F32, kind="ExternalInput")
    bk_d = nc.dram_tensor("bk", [DG], F32, kind="ExternalInput")
    bv_d = nc.dram_tensor("bv", [DG], BF16, kind="ExternalInput")
    maskt_d = nc.dram_tensor("maskt", [512, 512], BF16, kind="ExternalInput")
    out_d = nc.dram_tensor("out", [T, D], F32, kind="ExternalOutput")

    with tile.TileContext(nc) as tc:
        with nc.allow_low_precision(reason="bf16/fp8 matmul pipeline, fp32 psum"):
            _emit(nc, tc, x_d, wq_d, wk_d, wv_d, wo_d, bq_d, bk_d, bv_d,
                  maskt_d, out_d)
    nc.finalize()
    return nc


def _emit(nc, tc, x_d, wq_d, wk_d, wv_d, wo_d, bq_d, bk_d, bv_d, maskt_d,
          out_d):
    from contextlib import ExitStack
    ctx = ExitStack()
    with ctx:
        # ---------------- pools ----------------
        const_p = ctx.enter_context(tc.tile_pool(name="const", bufs=1))
        xt_p = ctx.enter_context(tc.tile_pool(name="xt", bufs=1))
        qk_p = ctx.enter_context(tc.tile_pool(name="qk", bufs=1))
        vp_p = ctx.enter_context(tc.tile_pool(name="vp", bufs=1))
        yt_p = ctx.enter_context(tc.tile_pool(name="yt", bufs=1))
        w_p = ctx.enter_context(tc.tile_pool(name="w", bufs=1))
        wblk_p = ctx.enter_context(tc.tile_pool(name="wblk", bufs=4))
        pt_p = ctx.enter_context(tc.tile_pool(name="pt", bufs=3))
        zt_p = ctx.enter_context(tc.tile_pool(name="zt", bufs=3))
        ob_p = ctx.enter_context(tc.tile_pool(name="ob", bufs=3))
        s_ps = ctx.enter_context(tc.tile_pool(name="s_ps", bufs=3, space="PSUM"))
        y_ps = ctx.enter_context(tc.tile_pool(name="y_ps", bufs=2, space="PSUM"))

        def sps():
            return s_ps.tile([128, 1024], F32, tag="s", name="sgrp")

        # ---------------- constants ----------------
        identb = const_p.tile([128, 128], BF16, tag="identb")
        make_identity(nc, identb[:])

        # e0b: row 0 ones (bf16, bias broadcast)
        e0b = const_p.tile([128, 128], BF16, tag="e0b")
        nc.vector.memset(e0b[:], 0.0)
        nc.vector.memset(e0b[0:1, :], 1.0)

        # esel_c[r, m] = (r == 4*(m//64) + c): selects zrec rows (c, 4+c) into
        # the two 64-row halves -> one matmul broadcasts 1/Z for both heads.
        esel = []
        for c in range(NC):
            et = const_p.tile([128, 128], BF16, tag=f"esel{c}", name=f"esel{c}")
            nc.gpsimd.memset(et[:], 1.0)
            nc.gpsimd.affine_select(
                out=et[:], in_=et[:], compare_op=ALU.is_equal, fill=0.0,
                base=-c, channel_multiplier=1, pattern=[[-4, 2], [0, 64]])
            esel.append(et)

        # q/k bias columns [128, 8]: cols 0-3 = bq tiles, 4-7 = bk tiles
        bqk = const_p.tile([128, 8], F32, tag="bqk")
        nc.sync.dma_start(bqk[:, 0:NK], bq_d[:].rearrange("(f p) -> p f", p=128))
        nc.sync.dma_start(bqk[:, NK:2 * NK], bk_d[:].rearrange("(f p) -> p f", p=128))

        # bv broadcast [128, 512]
        bvrow = const_p.tile([128, DG], BF16, tag="bvrow")
        nc.vector.memset(bvrow[:], 0.0)
        nc.sync.dma_start(bvrow[0:1, :], bv_d[:].rearrange("(o n) -> o n", o=1))
        bv_bc = const_p.tile([128, DG], BF16, tag="bv_bc")
        pb = sps()
        nc.tensor.matmul(pb[:, 0:DG], e0b[:], bvrow[:], start=True, stop=True)
        nc.vector.tensor_copy(bv_bc[:], pb[:, 0:DG])

        # zmat (Z rows, one tile per pair) / zrec (1/Z bf16, rows 0-7 live)
        zmat = [const_p.tile([8, 512], F32, tag=f"zmat{p}", name=f"zmat{p}")
                for p in range(NK)]
        zrec = const_p.tile([128, 512], BF16, tag="zrec")
        nc.vector.memset(zrec[:], 0.0)

        # transposed causal diag mask tiles (host pre-transposed)
        maskt = []
        for l in range(4):
            mt = const_p.tile([128, 512], BF16, tag=f"mt{l}", name=f"maskt{l}")
            nc.sync.dma_start(mt[:], maskt_d[128 * l:128 * (l + 1), :])
            maskt.append(mt)

        # exp bias column (fp8 range centering)
        ebias = const_p.tile([128, 1], F32, tag="ebias")
        nc.vector.memset(ebias[:], EBIAS)

        # ACT exp table preload (hide the ~2.7us table DMA in startup)
        dumm = const_p.tile([1, 16], F32, tag="dumm")
        nc.vector.memset(dumm[:], 0.0)
        nc.scalar.activation(dumm[:], dumm[:], AF.Exp, scale=1.0)

        # ---------------- xT (host pre-transposed, plain loads) ----------
        xt = [xt_p.tile([128, T], BF16, tag=f"xt{d}", name=f"xt{d}") for d in range(ND)]
        for d in range(ND):
            nc.sync.dma_start(xt[d][:], x_d[128 * d:128 * (d + 1), :])

        # ---------------- weights ----------------
        wv_sb = []
        for d in range(ND):
            wt = w_p.tile([128, DG], BF16, tag=f"wv{d}", name=f"wvt{d}")
            nc.sync.dma_start(wt[:], wv_d[128 * d:128 * (d + 1), :])
            wv_sb.append(wt)
        wo_sb = []
        for k in range(NK):
            wt = w_p.tile([128, D], BF16, tag=f"wo{k}", name=f"wot{k}")
            nc.sync.dma_start(wt[:], wo_d[128 * k:128 * (k + 1), :])
            wo_sb.append(wt)

        def load_wblk(is_k, f):
            src = wk_d if is_k else wq_d
            wblk = wblk_p.tile([128, D], BF16, tag="wblk")
            nc.sync.dma_start(
                wblk[:].rearrange("p (dt c) -> p dt c", dt=ND),
                src[:, 128 * f:128 * (f + 1)].rearrange("(dt p) c -> p dt c", p=128))
            return wblk

        # persistent SBUF tensors
        vp = []
        for t in range(NT):
            vt = vp_p.tile([128, HPG * VW], BF16, tag=f"vp{t}", name=f"vp{t}")
            nc.vector.memset(vt[:], 1.0)
            vp.append(vt)
        qk = [qk_p.tile([128, T], BF16, tag=f"qk{f}", name=f"qk{f}")
              for f in range(2 * NK)]
        yt = [yt_p.tile([128, T], BF16, tag=f"yt{k}", name=f"yt{k}")
              for k in range(NK)]

        # ---------------- v projection (plain, upfront) ----------------
        for t in range(NT):
            pv = sps()
            for d in range(ND):
                nc.tensor.matmul(pv[:, 0:DG], xt[d][:, 128 * t:128 * (t + 1)],
                                 wv_sb[d][:], start=(d == 0), stop=(d == ND - 1))
            nc.vector.tensor_tensor(
                vp[t][:].rearrange("p (h c) -> p h c", h=HPG)[:, :, 0:HD],
                pv[:, 0:DG].rearrange("p (h c) -> p h c", h=HPG),
                bv_bc[:].rearrange("p (h c) -> p h c", h=HPG),
                ALU.add)

        # qk projection piece generator: one f-block = 4 chunks x 8 d-matmuls,
        # yielded in 8 pieces of 4 matmuls (chunk halves), drain per chunk.
        # The psum tile is allocated when the first half RUNS (not at
        # generator-build time) so pool rotation order matches emission order.
        def qk_pieces(is_k, f):
            wblk = load_wblk(is_k, f)
            dst = qk[NK + f if is_k else f]
            bcol = NK + f if is_k else f
            state = {}
            for cidx in range(NC):
                for half in range(2):
                    def piece(wblk=wblk, cidx=cidx, half=half,
                              dst=dst, bcol=bcol):
                        if half == 0:
                            state[cidx] = sps()
                        pq = state[cidx]
                        for d in range(4 * half, 4 * half + 4):
                            nc.tensor.matmul(
                                pq[:, 0:512], wblk[:, 128 * d:128 * (d + 1)],
                                xt[d][:, 512 * cidx:512 * (cidx + 1)],
                                start=(d == 0), stop=(d == ND - 1))
                        if half == 1:
                            nc.vector.tensor_scalar(
                                dst[:, 512 * cidx:512 * (cidx + 1)],
                                pq[:, 0:512], bqk[:, bcol:bcol + 1], None,
                                ALU.add)
                    yield piece

        # qk pair 0 upfront (plain)
        for is_k in (False, True):
            for pc in qk_pieces(is_k, 0):
                pc()

        # ---------------- attention ----------------
        def norm_pieces(p):
            def recip(p=p):
                nc.vector.reciprocal(zrec[0:8, :], zmat[p][0:8, :])
            yield recip
            for c in range(NC):
                def piece(p=p, c=c):
                    zb = sps()
                    nc.tensor.matmul(zb[:, 0:512], esel[c][:], zrec[:],
                                     start=True, stop=True)
                    ysl = yt[p][:, 512 * c:512 * (c + 1)]
                    nc.vector.tensor_tensor(ysl, ysl, zb[:, 0:512], ALU.mult)
                yield piece

        filler_q = []  # queued PE filler pieces

        class Grp:
            __slots__ = ("p", "c", "j", "d", "offp", "ps", "pt", "py",
                         "chunk_end")

        def make_groups(p):
            gs = []
            for c in range(NC):
                for j in range(4 * c + 4):
                    g = Grp()
                    g.p, g.c, g.j = p, c, j
                    g.d = j - 4 * c
                    g.offp = 128 * g.d if g.d >= 0 else 0
                    g.chunk_end = (j == 4 * c + 3)
                    gs.append(g)
            return gs

        def emit_S(g):
            # head pair: even head at PE rows 0-63, odd at 64-127 -> the two
            # S matmuls land in distinct row groups and psum banks and run
            # concurrently (auto tile_position from base partitions).
            qt, kt = qk[g.p], qk[NK + g.p]
            g.ps = sps()
            for hl in range(2):
                qrow = 64 * hl
                nc.tensor.matmul(
                    g.ps[:, 512 * hl + g.offp:512 * (hl + 1)],
                    kt[qrow:qrow + HD, 128 * g.j:128 * (g.j + 1)],
                    qt[qrow:qrow + HD, 512 * g.c + g.offp:512 * (g.c + 1)],
                    start=True, stop=(g.d < 0), skip_group_check=True)
            if g.d >= 0:
                for hl in range(2):
                    nc.tensor.matmul(
                        g.ps[:, 512 * hl + g.offp:512 * (hl + 1)], identb[:],
                        maskt[g.d][:, g.offp:512],
                        start=False, stop=True, skip_group_check=True)

        def emit_EXP(g, pt_live):
            g.pt = pt_p.tile([128, 1024], BF16, tag="pt", name="pt")
            if g.offp:
                psv = g.ps[:].rearrange("p (two n) -> p two n", two=2)
                ptv = g.pt[:].rearrange("p (two n) -> p two n", two=2)
                nc.scalar.activation(ptv[:, :, g.offp:512],
                                     psv[:, :, g.offp:512],
                                     AF.Exp, scale=0.125)
            else:
                nc.scalar.activation(g.pt[:], g.ps[:], AF.Exp, scale=0.125)

        def emit_AV(g, py_live):
            if g.j == 0:
                py_live[0] = y_ps.tile([VW, 512], F32, tag="py", name="py0")
                py_live[1] = y_ps.tile([VW, 512], F32, tag="py", name="py1")
            g.py = (py_live[0], py_live[1])
            jmax = 4 * g.c + 3
            for hl in range(2):
                hidx = (2 * g.p + hl) % HPG
                nc.tensor.matmul(
                    g.py[hl][:, g.offp:512],
                    vp[g.j][:, VW * hidx:VW * hidx + VW],
                    g.pt[:, 512 * hl + g.offp:512 * (hl + 1)],
                    start=(g.j == 0), stop=(g.j == jmax),
                    skip_group_check=True)

        def emit_post(g):
            if not g.chunk_end:
                return
            for hl in range(2):
                qrow = 64 * hl
                # raw yT drain (normalized later) + Z row -> zmat
                nc.vector.tensor_copy(
                    yt[g.p][qrow:qrow + HD, 512 * g.c:512 * (g.c + 1)],
                    g.py[hl][0:64, :])
                i = 4 * hl + g.c
                zt = zt_p.tile([1, 512], F32, tag="zt", name="zt")
                nc.vector.tensor_copy(zt[:], g.py[hl][64:65, :])
                nc.sync.dma_start(zmat[g.p][i:i + 1, :], zt[:])

        for p in range(NK):
            groups = make_groups(p)
            # fillers: norm for pair p-1 first, then qk projections for p+1
            if p > 0:
                filler_q.extend(norm_pieces(p - 1))
            if p + 1 < NK:
                for is_k in (False, True):
                    filler_q.extend(qk_pieces(is_k, p + 1))
            prev = None
            py_live = [None, None]
            pt_live = [None]
            for g in groups:
                emit_S(g)
                emit_EXP(g, pt_live)
                if prev is not None:
                    emit_AV(prev, py_live)
                    emit_post(prev)
                # one filler piece per slot keeps PE ahead of ACT
                if filler_q:
                    filler_q.pop(0)()
                prev = g
            emit_AV(prev, py_live)
            emit_post(prev)

        # drain remaining fillers + last pair norm
        for pc in filler_q:
            pc()
        for pc in norm_pieces(NK - 1):
            pc()

        # ---------------- out projection ----------------
        for t in range(NT):
            po = sps()
            for oc in range(2):
                for k in range(NK):
                    nc.tensor.matmul(
                        po[:, 512 * oc:512 * (oc + 1)],
                        yt[k][:, 128 * t:128 * (t + 1)],
                        wo_sb[k][:, 512 * oc:512 * (oc + 1)],
                        start=(k == 0), stop=(k == NK - 1))
            ob = ob_p.tile([128, D], F32, tag="ob", name="ob")
            nc.vector.tensor_copy(ob[:, 0:512], po[:, 0:512])
            nc.scalar.copy(ob[:, 512:1024], po[:, 512:1024])
            nc.sync.dma_start(out_d[128 * t:128 * (t + 1), :], ob[:])


def kernel(x, attn_mask, Wqkv, bqkv, Wout, bout):
    if "nc" not in _cached:
        _cached["nc"] = _build()
    nc = _cached["nc"]

    x = np.asarray(x, dtype=np.float32)
    Wqkv = np.asarray(Wqkv, dtype=np.float32)
    bqkv = np.asarray(bqkv, dtype=np.float32)
    Wout = np.asarray(Wout, dtype=np.float32)
    bout = np.asarray(bout, dtype=np.float32)
    # transposed causal diagonal block, bf16
    maskt_blk = np.ascontiguousarray(
        np.asarray(attn_mask, dtype=np.float32)[0, 0, :512, :512].T
    ).astype(bfloat16)

    in_maps = []
    for b in range(B):
        for g in range(G):
            s = slice(g * DG, (g + 1) * DG)
            in_maps.append({
                "x": np.ascontiguousarray(x[b].T).astype(bfloat16),
                "wq": np.ascontiguousarray(Wqkv[:, g * DG:(g + 1) * DG]).astype(bfloat16),
                "wk": np.ascontiguousarray(Wqkv[:, D + g * DG:D + (g + 1) * DG]).astype(bfloat16),
                "wv": np.ascontiguousarray(Wqkv[:, 2 * D + g * DG:2 * D + (g + 1) * DG]).astype(bfloat16),
                "wo": np.ascontiguousarray(Wout[s, :]).astype(bfloat16),
                "bq": np.ascontiguousarray(bqkv[g * DG:(g + 1) * DG]),
                "bk": np.ascontiguousarray(bqkv[D + g * DG:D + (g + 1) * DG]),
                "bv": np.ascontiguousarray(bqkv[2 * D + g * DG:2 * D + (g + 1) * DG]).astype(bfloat16),
                "maskt": maskt_blk,
            })

    trace = bool(int(os.environ.get("BASS_ATTN_TRACE", "0")))
    res = bass_utils.run_bass_kernel_spmd(
        nc, in_maps, core_ids=list(range(B * G)), trace=trace)
    _cached["last_result"] = res

    out = np.empty((B, T, D), dtype=np.float32)
    for b in range(B):
        out[b] = res.results[2 * b]["out"] + res.results[2 * b + 1]["out"] \
            + bout[None, :]
    return out
